# revision 22
# baseline (speedup 1.0000x reference)
"""Distributed multi-head attention kernel for 8 Trainium2 NeuronCores.

Problem: x[2,2048,768] @ Wqkv[768,2304] + bqkv -> 12-head attention -> @ Wproj + bproj.

Sharding: data-parallel over batch (2) x sequence-quarter (4) = 8 cores.
Each core computes K/V for its whole batch (redundant across the 4 cores
sharing a batch -- avoids collectives, which measured slower), attention +
projection for its own 512 query rows.

All matmuls are full-width 128-contract / 128-out (partial-array matmuls run
at half the PE clock):
- The per-head S^T matmul (contract = head_dim = 64) is zero-padded via a
  per-head Q^T buffer whose complementary partition half is zeroed (zeros in
  the streaming operand kill the neighboring head's K rows in the stationary
  operand).
- The attention-value matmul reads a 128-wide window of the packed
  [V_0|1|V_1|1|...] buffer. Even heads use window [h*65, h*65+128): context
  rows 0:64, denominator (ones-column) row 64. Odd heads shift the window 64
  left: context lands on rows 64:128 and the PREVIOUS head's ones-column
  lands on row 63 -- still all-ones, so it yields this head's denominator.
  Junk output rows are never read.
- Head pairs therefore assemble a fully-packed context^T [128, 6, 512], and
  the projection contracts 128 real rows per pair (6 matmuls per output tile,
  no padding).

Softmax runs without max-subtraction (scores are O(1) for this data regime).
All 12 head denominators are gathered into one PSUM tile via one-hot matmuls
(host-provided one-hots at rows 63/64), inverted in a single DVE reciprocal,
broadcast back via selector matmuls, and multiplied into the context right
before projection (projection is linear, so normalize-late is exact).

Engine balance: zero-fills run on GpSimd, K^T/V evacuation on VectorE, Q^T
evacuation split ScalarE/VectorE, exp on ScalarE in 2-bank groups, and the
first two heads' score matmuls are emitted before the V projection so the
ScalarE exp pipeline starts early.
"""

import numpy as np
import ml_dtypes

B = 2
L = 2048
D = 768
H = 12
HD = 64
SCALE = HD ** -0.5
N_CORES = 8
LQ = L // 4  # 512 query rows per core

_CACHED = {}


def _build_nc():
    import concourse.bass as bass
    import concourse.mybir as mybir
    import concourse.tile as tile
    from concourse import bacc

    F32 = mybir.dt.float32
    BF16 = mybir.dt.bfloat16
    Alu = mybir.AluOpType
    Act = mybir.ActivationFunctionType

    nc = bacc.Bacc(target_bir_lowering=False)

    xT_h = nc.declare_dram_parameter("xT", [D, L], BF16, isOutput=False)
    xTq_h = nc.declare_dram_parameter("xTq", [D, LQ], BF16, isOutput=False)
    wqkv_h = nc.declare_dram_parameter("wqkv", [D, 3 * D], BF16, isOutput=False)
    bqkv_h = nc.declare_dram_parameter("bqkv", [3 * D], F32, isOutput=False)
    wp_h = nc.declare_dram_parameter("wproj2", [128, D // 128, D], BF16, isOutput=False)
    sel_h = nc.declare_dram_parameter("selmat", [H, H * 128], F32, isOutput=False)
    eye_h = nc.declare_dram_parameter("eyemat", [128, H * 128], F32, isOutput=False)
    bp_h = nc.declare_dram_parameter("bproj", [D], F32, isOutput=False)
    y_h = nc.declare_dram_parameter("y", [LQ, D], F32, isOutput=True)

    DT = D // 128      # 6 tiles of the qkv contraction dim
    LT = L // 128      # 16 key tiles
    JG = 2             # j-tiles per exp group (psum banks per S tile)
    VW = 65            # V block width per head (64 ctx + 1 ones)
    VPAD = 11 * VW + 128  # pad V free dim so lhsT can read 128 cols

    with tile.TileContext(nc) as tc:
        with tc.tile_pool(name="persist", bufs=1) as pp:
            # persistent across the whole kernel
            KT_sb = pp.tile([128, DT, L], BF16)         # K^T, plain [c, l] layout
            QTz_sb = pp.tile([128, H, LQ], BF16)        # Q^T per head, other parity half zeroed
            V_sb = pp.tile([128, LT, VPAD + 5], BF16)   # [V_h | ones] blocks at h*65
            OT2_sb = pp.tile([128, DT, LQ], BF16)       # ctx^T per head PAIR (fully packed)
            bias_sb = pp.tile([128, 18], F32)
            bv_sb = pp.tile([128, D], F32)
            eye_sb = pp.tile([128, H * 128], F32)       # gather one-hots at rows 63/64
            sel_sb = pp.tile([128, H * 128], F32)       # bcast selectors on rows 0:12, rest zero
            dst_sb = pp.tile([128, LQ], F32)            # denom staging, rows 63/64 live
            Rsb = pp.tile([128, LQ], F32)               # 1/denom on rows 0:12, rest zero
            Dsb = pp.tile([H, LQ], F32)

            # constants (zero-fills on the otherwise idle GpSimd engine)
            for h in range(H):
                nc.gpsimd.memset(QTz_sb[:, h, :], 0.0)
            nc.gpsimd.memset(dst_sb, 0.0)
            nc.gpsimd.memset(Rsb, 0.0)
            nc.vector.memset(sel_sb, 0.0)
            nc.sync.dma_start(out=sel_sb[0:H, :], in_=sel_h[:])
            nc.sync.dma_start(out=eye_sb, in_=eye_h[:])
            for h in range(H):
                nc.vector.memset(V_sb[:, :, h * VW + HD:h * VW + HD + 1], 1.0)

            nc.sync.dma_start(out=bias_sb, in_=bqkv_h[:].rearrange("(n p) -> p n", p=128))
            bv_src = bqkv_h[2 * D:3 * D]
            nc.gpsimd.dma_start(
                out=bv_sb,
                in_=bass.AP(tensor=bv_src.tensor, offset=bv_src.offset,
                            ap=[[0, 128]] + list(bv_src.ap)),
            )
            with (
                tc.tile_pool(name="loadp", bufs=1) as lp,
                tc.tile_pool(name="ps_s", bufs=2, space="PSUM") as ps_s,
                tc.tile_pool(name="ps_o", bufs=3, space="PSUM") as ps_o,
                tc.tile_pool(name="ps_d", bufs=1, space="PSUM") as ps_d,
                tc.tile_pool(name="ptp", bufs=2) as ptp,
            ):
                xT_sb = lp.tile([128, DT, L], BF16)
                xTq_sb = lp.tile([128, DT, LQ], BF16)
                wqkv_sb = lp.tile([128, DT, 3 * D], BF16)
                wp_sb = lp.tile([128, DT, D], BF16)
                bp_sb = lp.tile([128, D], F32)
                nc.sync.dma_start(out=wp_sb, in_=wp_h[:])
                bp_src = bp_h[:]
                nc.gpsimd.dma_start(
                    out=bp_sb,
                    in_=bass.AP(tensor=bp_src.tensor, offset=bp_src.offset,
                                ap=[[0, 128]] + list(bp_src.ap)),
                )

                wq_r = wqkv_h[:].rearrange("(n p) c -> p n c", p=128)
                nc.sync.dma_start(out=wqkv_sb[:, :, D:2 * D], in_=wq_r[:, :, D:2 * D])
                nc.sync.dma_start(out=xT_sb, in_=xT_h[:].rearrange("(n p) l -> p n l", p=128))
                nc.sync.dma_start(out=xTq_sb, in_=xTq_h[:].rearrange("(n p) l -> p n l", p=128))
                nc.sync.dma_start(out=wqkv_sb[:, :, 0:D], in_=wq_r[:, :, 0:D])
                nc.sync.dma_start(out=wqkv_sb[:, :, 2 * D:3 * D], in_=wq_r[:, :, 2 * D:3 * D])

                D_ps = ps_d.tile([128, LQ], F32)

                def ktqt_block(kt):
                    # K^T c-tile: single-op evac with per-partition bias
                    for lc in range(4):
                        ps = ps_s.tile([128, JG, LQ], F32, tag="sps")
                        for dt in range(DT):
                            nc.tensor.matmul(
                                ps[:, 0, :],
                                wqkv_sb[:, dt, D + kt * 128:D + (kt + 1) * 128],
                                xT_sb[:, dt, lc * 512:(lc + 1) * 512],
                                start=(dt == 0), stop=(dt == DT - 1),
                            )
                        nc.vector.tensor_scalar_add(
                            KT_sb[:, kt, lc * 512:(lc + 1) * 512], ps[:, 0, :],
                            bias_sb[:, 6 + kt:7 + kt])
                    # Q^T c-tile: evac halves split into the per-head
                    # zero-padded layout (ScalarE / VectorE)
                    ps = ps_s.tile([128, JG, LQ], F32, tag="sps")
                    for dt in range(DT):
                        nc.tensor.matmul(
                            ps[:, 0, :],
                            wqkv_sb[:, dt, kt * 128:(kt + 1) * 128],
                            xTq_sb[:, dt, :],
                            start=(dt == 0), stop=(dt == DT - 1),
                        )
                    nc.scalar.activation(
                        QTz_sb[0:64, 2 * kt, :], ps[0:64, 0, :],
                        Act.Identity, bias=bias_sb[0:64, kt:kt + 1])
                    nc.vector.tensor_scalar_add(
                        QTz_sb[64:128, 2 * kt + 1, :], ps[64:128, 0, :],
                        bias_sb[64:128, kt:kt + 1])

                def v_block(lt):
                    for hf in range(2):
                        ps = ps_o.tile([128, LQ], F32, tag="ops")
                        for dt in range(DT):
                            nc.tensor.matmul(
                                ps[:, :384],
                                xT_sb[:, dt, lt * 128:(lt + 1) * 128],
                                wqkv_sb[:, dt, 2 * D + hf * 384:2 * D + (hf + 1) * 384],
                                start=(dt == 0), stop=(dt == DT - 1),
                            )
                        nc.vector.tensor_tensor(
                            V_sb[:, lt, 390 * hf:390 * hf + 390].rearrange(
                                "p (h c) -> p h c", c=VW)[:, :, 0:HD],
                            ps[:, :384].rearrange("p (h d) -> p h d", h=6),
                            bv_sb[:, hf * 384:(hf + 1) * 384].rearrange(
                                "p (h d) -> p h d", h=6),
                            Alu.add,
                        )

                def s_block(h):
                    # S^T[j, i] = sum_d K^T[d, j] Q^T[d, i]; zeros in QTz kill
                    # the neighboring head's K rows
                    PT = ptp.tile([128, LT, LQ], BF16, tag="PT")
                    for g in range(LT // JG):
                        sps = ps_s.tile([128, JG, LQ], F32, tag="sps")
                        for t in range(JG):
                            jt = JG * g + t
                            nc.tensor.matmul(
                                sps[:, t, :],
                                KT_sb[:, h // 2, jt * 128:(jt + 1) * 128],
                                QTz_sb[:, h, :],
                                start=True, stop=True,
                            )
                        nc.scalar.activation(
                            PT[:, JG * g:JG * (g + 1), :], sps, Act.Exp,
                            scale=SCALE)
                    return PT

                def out_block(h, PT):
                    # out^T = sum_j [..V_h|1..][j, :] P^T[j, i] through a
                    # 128-wide window: even heads ctx rows 0:64 + denom row 64,
                    # odd heads (window shifted 64 left) ctx rows 64:128 +
                    # denom row 63 (previous head's ones-column)
                    p0 = (h % 2) * 64
                    dr = 64 - (h % 2)
                    voff = h * VW - p0
                    ops = ps_o.tile([128, LQ], F32, tag="ops")
                    for jt in range(LT):
                        nc.tensor.matmul(
                            ops,
                            V_sb[:, jt, voff:voff + 128],
                            PT[:, jt, :],
                            start=(jt == 0), stop=(jt == LT - 1),
                        )
                    nc.vector.tensor_copy(
                        OT2_sb[p0:p0 + 64, h // 2, :], ops[p0:p0 + 64, :])
                    if h % 2 == 0:
                        nc.vector.tensor_copy(dst_sb[64:65, :], ops[64:65, :])
                    else:
                        nc.vector.tensor_copy(dst_sb[32:64, :], ops[32:64, :])
                    nc.tensor.matmul(
                        D_ps, eye_sb[:, h * 128:(h + 1) * 128], dst_sb,
                        start=(h == 0), stop=(h == H - 1),
                        skip_group_check=True,
                    )

                # normalization (reciprocal + bcast + mult)
                def normalize(h0, h1):
                    nc.vector.tensor_copy(Dsb[0:h1, :], D_ps[0:H, :][0:h1, :])
                    nc.vector.reciprocal(out=Rsb[0:h1, :], in_=Dsb[0:h1, :])
                    for h in range(h0, h1):
                        p0 = (h % 2) * 64
                        rb = ps_s.tile([128, LQ], F32, tag="sps")
                        nc.tensor.matmul(
                            rb, sel_sb[:, h * 128:(h + 1) * 128], Rsb,
                            start=True, stop=True)
                        nc.vector.tensor_tensor(
                            OT2_sb[p0:p0 + 64, h // 2, :],
                            OT2_sb[p0:p0 + 64, h // 2, :], rb[p0:p0 + 64, :],
                            Alu.mult)

                # ---- schedule: K^T/Q^T, first two heads' scores, V, rest ----
                for kt in range(DT):
                    ktqt_block(kt)
                PT0 = s_block(0)
                PT1 = s_block(1)
                for lt in range(LT):
                    v_block(lt)
                out_block(0, PT0)
                out_block(1, PT1)
                for h in range(2, H):
                    PT = s_block(h)
                    out_block(h, PT)
                normalize(0, H)

                # ---- projection, interleaved with normalization via deps ----
                with tc.tile_pool(name="yp", bufs=2) as yp:
                    y_r = y_h[:].rearrange("(n p) e -> p n e", p=128)
                    for ic in range(LQ // 128):
                        for eh in range(2):
                            ps = ps_o.tile([128, LQ], F32, tag="ops")
                            for pt in range(DT):
                                nc.tensor.matmul(
                                    ps[:, :384],
                                    OT2_sb[:, pt, ic * 128:(ic + 1) * 128],
                                    wp_sb[:, pt, eh * 384:(eh + 1) * 384],
                                    start=(pt == 0), stop=(pt == DT - 1),
                                )
                            yt = yp.tile([128, 384], F32)
                            nc.vector.tensor_tensor(
                                yt, ps[:, :384], bp_sb[:, eh * 384:(eh + 1) * 384],
                                Alu.add)
                            nc.sync.dma_start(
                                out=y_r[:, ic, eh * 384:(eh + 1) * 384], in_=yt)

    nc.finalize()
    return nc


def _get_nc():
    if "nc" not in _CACHED:
        _CACHED["nc"] = _build_nc()
    return _CACHED["nc"]


def _make_in_maps(x, Wqkv, bqkv, Wproj, bproj):
    bf16 = ml_dtypes.bfloat16
    x = np.asarray(x, dtype=np.float32)
    wqkv16 = np.ascontiguousarray(np.asarray(Wqkv, dtype=np.float32).astype(bf16))
    bqkv32 = np.ascontiguousarray(np.asarray(bqkv, dtype=np.float32))
    wp2 = np.ascontiguousarray(
        np.asarray(Wproj, dtype=np.float32).astype(bf16)
        .reshape(D // 128, 128, D).transpose(1, 0, 2))
    bp32 = np.ascontiguousarray(np.asarray(bproj, dtype=np.float32))
    selmat = np.zeros((H, H * 128), np.float32)
    for h in range(H):
        selmat[h, h * 128:(h + 1) * 128] = 1.0
    eyemat = np.zeros((128, H * 128), np.float32)
    for h in range(H):
        eyemat[64 - (h % 2), h * 128 + h] = 1.0

    xT = [np.ascontiguousarray(x[b].T.astype(bf16)) for b in range(B)]
    in_maps = []
    for c in range(N_CORES):
        b, s = c // 4, c % 4
        in_maps.append({
            "xT": xT[b],
            "xTq": np.ascontiguousarray(xT[b][:, s * LQ:(s + 1) * LQ]),
            "wqkv": wqkv16,
            "bqkv": bqkv32,
            "wproj2": wp2,
            "bproj": bp32,
            "selmat": selmat,
            "eyemat": eyemat,
        })
    return in_maps


def run(inputs, trace=False):
    """Run the SPMD kernel. Returns (full_output [2,2048,768] f32, BassKernelResults)."""
    from concourse.bass_utils import run_bass_kernel_spmd

    nc = _get_nc()
    in_maps = _make_in_maps(**inputs)
    res = run_bass_kernel_spmd(nc, in_maps, list(range(N_CORES)), trace=trace)
    out = np.empty((B, L, D), dtype=np.float32)
    for c in range(N_CORES):
        b, s = c // 4, c % 4
        out[b, s * LQ:(s + 1) * LQ, :] = res.results[c]["y"]
    return out, res


def kernel(**inputs) -> np.ndarray:
    return run(inputs)[0]


# revision 24
# speedup vs baseline: 1.0310x; 1.0310x over previous
"""Distributed multi-head attention kernel for 8 Trainium2 NeuronCores.

Problem: x[2,2048,768] @ Wqkv[768,2304] + bqkv -> 12-head attention -> @ Wproj + bproj.

Sharding: data-parallel over batch (2) x sequence-quarter (4) = 8 cores.
Each core computes K/V for its whole batch (redundant across the 4 cores
sharing a batch -- avoids collectives, which measured slower), attention +
projection for its own 512 query rows.

All matmuls are full-width 128-contract / 128-out (partial-array matmuls run
at half the PE clock):
- The per-head S^T matmul (contract = head_dim = 64) is zero-padded via a
  per-head Q^T buffer whose complementary partition half is zeroed (zeros in
  the streaming operand kill the neighboring head's K rows in the stationary
  operand).
- The attention-value matmul reads a 128-wide window of the packed
  [V_0|1|V_1|1|...] buffer. Even heads use window [h*65, h*65+128): context
  rows 0:64, denominator (ones-column) row 64. Odd heads shift the window 64
  left: context lands on rows 64:128 and the PREVIOUS head's ones-column
  lands on row 63 -- still all-ones, so it yields this head's denominator.
  Junk output rows are never read.
- Head pairs therefore assemble a fully-packed context^T [128, 6, 512], and
  the projection contracts 128 real rows per pair (6 matmuls per output tile,
  no padding).

Softmax runs without max-subtraction (scores are O(1) for this data regime).
All 12 head denominators are gathered into one PSUM tile via one-hot matmuls
(host-provided one-hots at rows 63/64), inverted in a single DVE reciprocal,
broadcast back via selector matmuls, and multiplied into the context right
before projection (projection is linear, so normalize-late is exact).

Engine balance: zero-fills run on GpSimd, K^T/V evacuation on VectorE, Q^T
evacuation split ScalarE/VectorE, exp on ScalarE in 2-bank groups, and the
first two heads' score matmuls are emitted before the V projection so the
ScalarE exp pipeline starts early.
"""

import numpy as np
import ml_dtypes

B = 2
L = 2048
D = 768
H = 12
HD = 64
SCALE = HD ** -0.5
N_CORES = 8
LQ = L // 4  # 512 query rows per core

_CACHED = {}


def _build_nc():
    import concourse.bass as bass
    import concourse.mybir as mybir
    import concourse.tile as tile
    from concourse import bacc

    F32 = mybir.dt.float32
    BF16 = mybir.dt.bfloat16
    Alu = mybir.AluOpType
    Act = mybir.ActivationFunctionType

    nc = bacc.Bacc(target_bir_lowering=False)

    xT_h = nc.declare_dram_parameter("xT", [D, L], BF16, isOutput=False)
    xTq_h = nc.declare_dram_parameter("xTq", [D, LQ], BF16, isOutput=False)
    wqkv_h = nc.declare_dram_parameter("wqkv", [D, 3 * D], BF16, isOutput=False)
    bqkv_h = nc.declare_dram_parameter("bqkv", [3 * D], F32, isOutput=False)
    wp_h = nc.declare_dram_parameter("wproj2", [128, D // 128, D], BF16, isOutput=False)
    sel_h = nc.declare_dram_parameter("selmat", [H, H * 128], BF16, isOutput=False)
    eye_h = nc.declare_dram_parameter("eyemat", [128, H * 128], F32, isOutput=False)
    bp_h = nc.declare_dram_parameter("bproj", [D], F32, isOutput=False)
    y_h = nc.declare_dram_parameter("y", [LQ, D], F32, isOutput=True)

    DT = D // 128      # 6 tiles of the qkv contraction dim
    LT = L // 128      # 16 key tiles
    JG = 2             # j-tiles per exp group (psum banks per S tile)
    VW = 65            # V block width per head (64 ctx + 1 ones)
    VPAD = 11 * VW + 128  # pad V free dim so lhsT can read 128 cols

    with tile.TileContext(nc) as tc:
        with tc.tile_pool(name="persist", bufs=1) as pp:
            # persistent across the whole kernel
            KT_sb = pp.tile([128, DT, L], BF16)         # K^T, plain [c, l] layout
            QTz_sb = pp.tile([128, H, LQ], BF16)        # Q^T per head, other parity half zeroed
            V_sb = pp.tile([128, LT, VPAD + 5], BF16)   # [V_h | ones] blocks at h*65
            OT2_sb = pp.tile([128, DT, LQ], BF16)       # ctx^T per head PAIR (fully packed)
            bias_sb = pp.tile([128, 18], F32)
            bv_sb = pp.tile([128, D], F32)
            eye_sb = pp.tile([128, H * 128], F32)       # gather one-hots at rows 63/64
            sel_sb = pp.tile([128, H * 128], BF16)      # bcast selectors on rows 0:12, rest zero
            R16 = pp.tile([128, LQ], BF16)              # bf16 1/denom, rows 12:128 zero
            dst_sb = pp.tile([128, LQ], F32)            # denom staging, rows 63/64 live
            Rsb = pp.tile([128, LQ], F32)               # 1/denom on rows 0:12, rest zero
            Dsb = pp.tile([H, LQ], F32)

            # constants (zero-fills on the otherwise idle GpSimd engine)
            for h in range(H):
                nc.gpsimd.memset(QTz_sb[:, h, :], 0.0)
            nc.gpsimd.memset(dst_sb, 0.0)
            nc.gpsimd.memset(Rsb, 0.0)
            nc.gpsimd.memset(R16, 0.0)
            nc.vector.memset(sel_sb, 0.0)
            nc.sync.dma_start(out=sel_sb[0:H, :], in_=sel_h[:])
            nc.sync.dma_start(out=eye_sb, in_=eye_h[:])
            for h in range(H):
                nc.vector.memset(V_sb[:, :, h * VW + HD:h * VW + HD + 1], 1.0)

            nc.sync.dma_start(out=bias_sb, in_=bqkv_h[:].rearrange("(n p) -> p n", p=128))
            bv_src = bqkv_h[2 * D:3 * D]
            nc.gpsimd.dma_start(
                out=bv_sb,
                in_=bass.AP(tensor=bv_src.tensor, offset=bv_src.offset,
                            ap=[[0, 128]] + list(bv_src.ap)),
            )
            with (
                tc.tile_pool(name="loadp", bufs=1) as lp,
                tc.tile_pool(name="ps_s", bufs=2, space="PSUM") as ps_s,
                tc.tile_pool(name="ps_o", bufs=3, space="PSUM") as ps_o,
                tc.tile_pool(name="ps_d", bufs=1, space="PSUM") as ps_d,
                tc.tile_pool(name="ptp", bufs=2) as ptp,
            ):
                xT_sb = lp.tile([128, DT, L], BF16)
                xTq_sb = lp.tile([128, DT, LQ], BF16)
                wqkv_sb = lp.tile([128, DT, 3 * D], BF16)
                wp_sb = lp.tile([128, DT, D], BF16)
                bp_sb = lp.tile([128, D], F32)

                wq_r = wqkv_h[:].rearrange("(n p) c -> p n c", p=128)
                nc.sync.dma_start(out=wqkv_sb[:, :, D:2 * D], in_=wq_r[:, :, D:2 * D])
                nc.sync.dma_start(out=xT_sb, in_=xT_h[:].rearrange("(n p) l -> p n l", p=128))
                nc.sync.dma_start(out=xTq_sb, in_=xTq_h[:].rearrange("(n p) l -> p n l", p=128))
                nc.sync.dma_start(out=wqkv_sb[:, :, 0:D], in_=wq_r[:, :, 0:D])
                nc.sync.dma_start(out=wqkv_sb[:, :, 2 * D:3 * D], in_=wq_r[:, :, 2 * D:3 * D])
                nc.sync.dma_start(out=wp_sb, in_=wp_h[:])
                bp_src = bp_h[:]
                nc.gpsimd.dma_start(
                    out=bp_sb,
                    in_=bass.AP(tensor=bp_src.tensor, offset=bp_src.offset,
                                ap=[[0, 128]] + list(bp_src.ap)),
                )

                D_ps = ps_d.tile([128, LQ], F32)

                def ktqt_block(kt):
                    # K^T c-tile: single-op evac with per-partition bias
                    for lc in range(4):
                        ps = ps_s.tile([128, JG, LQ], F32, tag="sps")
                        for dt in range(DT):
                            nc.tensor.matmul(
                                ps[:, 0, :],
                                wqkv_sb[:, dt, D + kt * 128:D + (kt + 1) * 128],
                                xT_sb[:, dt, lc * 512:(lc + 1) * 512],
                                start=(dt == 0), stop=(dt == DT - 1),
                            )
                        nc.vector.tensor_scalar_add(
                            KT_sb[:, kt, lc * 512:(lc + 1) * 512], ps[:, 0, :],
                            bias_sb[:, 6 + kt:7 + kt])
                    # Q^T c-tile: evac halves split into the per-head
                    # zero-padded layout (ScalarE / VectorE)
                    ps = ps_s.tile([128, JG, LQ], F32, tag="sps")
                    for dt in range(DT):
                        nc.tensor.matmul(
                            ps[:, 0, :],
                            wqkv_sb[:, dt, kt * 128:(kt + 1) * 128],
                            xTq_sb[:, dt, :],
                            start=(dt == 0), stop=(dt == DT - 1),
                        )
                    nc.scalar.activation(
                        QTz_sb[0:64, 2 * kt, :], ps[0:64, 0, :],
                        Act.Identity, bias=bias_sb[0:64, kt:kt + 1])
                    nc.vector.tensor_scalar_add(
                        QTz_sb[64:128, 2 * kt + 1, :], ps[64:128, 0, :],
                        bias_sb[64:128, kt:kt + 1])

                def v_block(lt):
                    for hf in range(2):
                        ps = ps_o.tile([128, LQ], F32, tag="ops")
                        for dt in range(DT):
                            nc.tensor.matmul(
                                ps[:, :384],
                                xT_sb[:, dt, lt * 128:(lt + 1) * 128],
                                wqkv_sb[:, dt, 2 * D + hf * 384:2 * D + (hf + 1) * 384],
                                start=(dt == 0), stop=(dt == DT - 1),
                            )
                        nc.vector.tensor_tensor(
                            V_sb[:, lt, 390 * hf:390 * hf + 390].rearrange(
                                "p (h c) -> p h c", c=VW)[:, :, 0:HD],
                            ps[:, :384].rearrange("p (h d) -> p h d", h=6),
                            bv_sb[:, hf * 384:(hf + 1) * 384].rearrange(
                                "p (h d) -> p h d", h=6),
                            Alu.add,
                        )

                def s_block(h):
                    # S^T[j, i] = sum_d K^T[d, j] Q^T[d, i]; zeros in QTz kill
                    # the neighboring head's K rows
                    PT = ptp.tile([128, LT, LQ], BF16, tag="PT")
                    for g in range(LT // JG):
                        sps = ps_s.tile([128, JG, LQ], F32, tag="sps")
                        for t in range(JG):
                            jt = JG * g + t
                            nc.tensor.matmul(
                                sps[:, t, :],
                                KT_sb[:, h // 2, jt * 128:(jt + 1) * 128],
                                QTz_sb[:, h, :],
                                start=True, stop=True,
                            )
                        nc.scalar.activation(
                            PT[:, JG * g:JG * (g + 1), :], sps, Act.Exp,
                            scale=SCALE)
                    return PT

                def out_block(h, PT):
                    # out^T = sum_j [..V_h|1..][j, :] P^T[j, i] through a
                    # 128-wide window: even heads ctx rows 0:64 + denom row 64,
                    # odd heads (window shifted 64 left) ctx rows 64:128 +
                    # denom row 63 (previous head's ones-column)
                    p0 = (h % 2) * 64
                    dr = 64 - (h % 2)
                    voff = h * VW - p0
                    ops = ps_o.tile([128, LQ], F32, tag="ops")
                    for jt in range(LT):
                        nc.tensor.matmul(
                            ops,
                            V_sb[:, jt, voff:voff + 128],
                            PT[:, jt, :],
                            start=(jt == 0), stop=(jt == LT - 1),
                        )
                    nc.vector.tensor_copy(
                        OT2_sb[p0:p0 + 64, h // 2, :], ops[p0:p0 + 64, :])
                    if h % 2 == 0:
                        nc.vector.tensor_copy(dst_sb[64:65, :], ops[64:65, :])
                    else:
                        nc.vector.tensor_copy(dst_sb[32:64, :], ops[32:64, :])
                    nc.tensor.matmul(
                        D_ps, eye_sb[:, h * 128:(h + 1) * 128], dst_sb,
                        start=(h == 0), stop=(h == H - 1),
                        skip_group_check=True,
                    )

                # normalization (reciprocal + bcast + mult)
                def normalize(h0, h1):
                    nc.vector.tensor_copy(Dsb[0:h1, :], D_ps[0:H, :][0:h1, :])
                    nc.vector.reciprocal(out=Rsb[0:h1, :], in_=Dsb[0:h1, :])
                    nc.vector.tensor_copy(R16[0:h1, :], Rsb[0:h1, :])
                    for h in range(h0, h1):
                        p0 = (h % 2) * 64
                        rb = ps_s.tile([128, LQ], F32, tag="sps")
                        nc.tensor.matmul(
                            rb, sel_sb[:, h * 128:(h + 1) * 128], R16,
                            start=True, stop=True)
                        nc.vector.tensor_tensor(
                            OT2_sb[p0:p0 + 64, h // 2, :],
                            OT2_sb[p0:p0 + 64, h // 2, :], rb[p0:p0 + 64, :],
                            Alu.mult)

                # ---- schedule: K^T/Q^T, first two heads' scores, V, rest ----
                for kt in range(DT):
                    ktqt_block(kt)
                PT0 = s_block(0)
                PT1 = s_block(1)
                for lt in range(LT):
                    v_block(lt)
                out_block(0, PT0)
                out_block(1, PT1)
                for h in range(2, H):
                    PT = s_block(h)
                    out_block(h, PT)
                normalize(0, H)

                # ---- projection, interleaved with normalization via deps ----
                with tc.tile_pool(name="yp", bufs=2) as yp:
                    y_r = y_h[:].rearrange("(n p) e -> p n e", p=128)
                    for ic in range(LQ // 128):
                        for eh in range(2):
                            ps = ps_o.tile([128, LQ], F32, tag="ops")
                            for pt in range(DT):
                                nc.tensor.matmul(
                                    ps[:, :384],
                                    OT2_sb[:, pt, ic * 128:(ic + 1) * 128],
                                    wp_sb[:, pt, eh * 384:(eh + 1) * 384],
                                    start=(pt == 0), stop=(pt == DT - 1),
                                )
                            yt = yp.tile([128, 384], F32)
                            nc.vector.tensor_tensor(
                                yt, ps[:, :384], bp_sb[:, eh * 384:(eh + 1) * 384],
                                Alu.add)
                            nc.sync.dma_start(
                                out=y_r[:, ic, eh * 384:(eh + 1) * 384], in_=yt)

    nc.finalize()
    return nc


def _get_nc():
    if "nc" not in _CACHED:
        _CACHED["nc"] = _build_nc()
    return _CACHED["nc"]


def _make_in_maps(x, Wqkv, bqkv, Wproj, bproj):
    bf16 = ml_dtypes.bfloat16
    x = np.asarray(x, dtype=np.float32)
    wqkv16 = np.ascontiguousarray(np.asarray(Wqkv, dtype=np.float32).astype(bf16))
    bqkv32 = np.ascontiguousarray(np.asarray(bqkv, dtype=np.float32))
    wp2 = np.ascontiguousarray(
        np.asarray(Wproj, dtype=np.float32).astype(bf16)
        .reshape(D // 128, 128, D).transpose(1, 0, 2))
    bp32 = np.ascontiguousarray(np.asarray(bproj, dtype=np.float32))
    selmat = np.zeros((H, H * 128), ml_dtypes.bfloat16)
    for h in range(H):
        selmat[h, h * 128:(h + 1) * 128] = 1.0
    eyemat = np.zeros((128, H * 128), np.float32)
    for h in range(H):
        eyemat[64 - (h % 2), h * 128 + h] = 1.0

    xT = [np.ascontiguousarray(x[b].T.astype(bf16)) for b in range(B)]
    in_maps = []
    for c in range(N_CORES):
        b, s = c // 4, c % 4
        in_maps.append({
            "xT": xT[b],
            "xTq": np.ascontiguousarray(xT[b][:, s * LQ:(s + 1) * LQ]),
            "wqkv": wqkv16,
            "bqkv": bqkv32,
            "wproj2": wp2,
            "bproj": bp32,
            "selmat": selmat,
            "eyemat": eyemat,
        })
    return in_maps


def run(inputs, trace=False):
    """Run the SPMD kernel. Returns (full_output [2,2048,768] f32, BassKernelResults)."""
    from concourse.bass_utils import run_bass_kernel_spmd

    nc = _get_nc()
    in_maps = _make_in_maps(**inputs)
    res = run_bass_kernel_spmd(nc, in_maps, list(range(N_CORES)), trace=trace)
    out = np.empty((B, L, D), dtype=np.float32)
    for c in range(N_CORES):
        b, s = c // 4, c % 4
        out[b, s * LQ:(s + 1) * LQ, :] = res.results[c]["y"]
    return out, res


def kernel(**inputs) -> np.ndarray:
    return run(inputs)[0]


# revision 25
# speedup vs baseline: 1.0723x; 1.0400x over previous
"""Distributed multi-head attention kernel for 8 Trainium2 NeuronCores.

Problem: x[2,2048,768] @ Wqkv[768,2304] + bqkv -> 12-head attention -> @ Wproj + bproj.

Sharding: data-parallel over batch (2) x sequence-quarter (4) = 8 cores.
Each core computes K/V for its whole batch (redundant across the 4 cores
sharing a batch -- avoids collectives, which measured slower), attention +
projection for its own 512 query rows.

All matmuls are full-width 128-contract / 128-out (partial-array matmuls run
at half the PE clock):
- The per-head S^T matmul (contract = head_dim = 64) is zero-padded via a
  per-head Q^T buffer whose complementary partition half is zeroed (zeros in
  the streaming operand kill the neighboring head's K rows in the stationary
  operand).
- The attention-value matmul reads a 128-wide window of the packed
  [V_0|1|V_1|1|...] buffer. Even heads use window [h*65, h*65+128): context
  rows 0:64, denominator (ones-column) row 64. Odd heads shift the window 64
  left: context lands on rows 64:128 and the PREVIOUS head's ones-column
  lands on row 63 -- still all-ones, so it yields this head's denominator.
  Junk output rows are never read.
- Head pairs therefore assemble a fully-packed context^T [128, 6, 512], and
  the projection contracts 128 real rows per pair (6 matmuls per output tile,
  no padding).

Softmax runs without max-subtraction (scores are O(1) for this data regime).
All 12 head denominators are gathered into one PSUM tile via one-hot matmuls
(host-provided one-hots at rows 63/64), inverted in a single DVE reciprocal,
broadcast back via selector matmuls, and multiplied into the context right
before projection (projection is linear, so normalize-late is exact).

Engine balance: zero-fills run on GpSimd, K^T/V evacuation on VectorE, Q^T
evacuation split ScalarE/VectorE, exp on ScalarE in 2-bank groups, and the
first two heads' score matmuls are emitted before the V projection so the
ScalarE exp pipeline starts early.
"""

import numpy as np
import ml_dtypes

B = 2
L = 2048
D = 768
H = 12
HD = 64
SCALE = HD ** -0.5
N_CORES = 8
LQ = L // 4  # 512 query rows per core

_CACHED = {}


def _build_nc():
    import concourse.bass as bass
    import concourse.mybir as mybir
    import concourse.tile as tile
    from concourse import bacc

    F32 = mybir.dt.float32
    BF16 = mybir.dt.bfloat16
    Alu = mybir.AluOpType
    Act = mybir.ActivationFunctionType

    nc = bacc.Bacc(target_bir_lowering=False)

    xT_h = nc.declare_dram_parameter("xT", [D, L], BF16, isOutput=False)
    xTq_h = nc.declare_dram_parameter("xTq", [D, LQ], BF16, isOutput=False)
    wqkv_h = nc.declare_dram_parameter("wqkv", [D, 3 * D], BF16, isOutput=False)
    bqkv_h = nc.declare_dram_parameter("bqkv", [3 * D], F32, isOutput=False)
    wp_h = nc.declare_dram_parameter("wproj2", [128, D // 128, D], BF16, isOutput=False)
    sel_h = nc.declare_dram_parameter("selmat", [H, H * 128], BF16, isOutput=False)
    eye_h = nc.declare_dram_parameter("eyemat", [128, H * 128], BF16, isOutput=False)
    bp_h = nc.declare_dram_parameter("bproj", [D], F32, isOutput=False)
    y_h = nc.declare_dram_parameter("y", [LQ, D], F32, isOutput=True)

    DT = D // 128      # 6 tiles of the qkv contraction dim
    LT = L // 128      # 16 key tiles
    JG = 2             # j-tiles per exp group (psum banks per S tile)
    VW = 65            # V block width per head (64 ctx + 1 ones)
    VPAD = 11 * VW + 128  # pad V free dim so lhsT can read 128 cols

    with tile.TileContext(nc) as tc:
        with tc.tile_pool(name="persist", bufs=1) as pp:
            # persistent across the whole kernel
            KT_sb = pp.tile([128, DT, L], BF16)         # K^T, plain [c, l] layout
            QTz_sb = pp.tile([128, H, LQ], BF16)        # Q^T per head, other parity half zeroed
            V_sb = pp.tile([128, LT, VPAD + 5], BF16)   # [V_h | ones] blocks at h*65
            OT2_sb = pp.tile([128, DT, LQ], BF16)       # ctx^T per head PAIR (fully packed)
            bias_sb = pp.tile([128, 18], F32)
            bv_sb = pp.tile([128, D], F32)
            eye_sb = pp.tile([128, H * 128], BF16)      # gather one-hots at rows 63/64
            sel_sb = pp.tile([128, H * 128], BF16)      # bcast selectors on rows 0:12, rest zero
            R16 = pp.tile([128, LQ], BF16)              # bf16 1/denom, rows 12:128 zero
            dst_sb = pp.tile([128, LQ], BF16)           # denom staging, rows 63/64 live
            Rsb = pp.tile([128, LQ], F32)               # 1/denom on rows 0:12, rest zero
            Dsb = pp.tile([H, LQ], F32)

            # constants (zero-fills on the otherwise idle GpSimd engine)
            for h in range(H):
                nc.gpsimd.memset(QTz_sb[:, h, :], 0.0)
            nc.gpsimd.memset(dst_sb, 0.0)
            nc.gpsimd.memset(Rsb, 0.0)
            nc.gpsimd.memset(R16, 0.0)
            nc.vector.memset(sel_sb, 0.0)
            nc.sync.dma_start(out=sel_sb[0:H, :], in_=sel_h[:])
            nc.sync.dma_start(out=eye_sb, in_=eye_h[:])
            for h in range(H):
                nc.vector.memset(V_sb[:, :, h * VW + HD:h * VW + HD + 1], 1.0)

            nc.sync.dma_start(out=bias_sb, in_=bqkv_h[:].rearrange("(n p) -> p n", p=128))
            bv_src = bqkv_h[2 * D:3 * D]
            nc.gpsimd.dma_start(
                out=bv_sb,
                in_=bass.AP(tensor=bv_src.tensor, offset=bv_src.offset,
                            ap=[[0, 128]] + list(bv_src.ap)),
            )
            with (
                tc.tile_pool(name="loadp", bufs=1) as lp,
                tc.tile_pool(name="ps_s", bufs=2, space="PSUM") as ps_s,
                tc.tile_pool(name="ps_o", bufs=3, space="PSUM") as ps_o,
                tc.tile_pool(name="ps_d", bufs=1, space="PSUM") as ps_d,
                tc.tile_pool(name="ptp", bufs=2) as ptp,
            ):
                xT_sb = lp.tile([128, DT, L], BF16)
                xTq_sb = lp.tile([128, DT, LQ], BF16)
                wqkv_sb = lp.tile([128, DT, 3 * D], BF16)
                wp_sb = lp.tile([128, DT, D], BF16)
                bp_sb = lp.tile([128, D], F32)

                wq_r = wqkv_h[:].rearrange("(n p) c -> p n c", p=128)
                nc.sync.dma_start(out=wqkv_sb[:, :, D:2 * D], in_=wq_r[:, :, D:2 * D])
                xT_r = xT_h[:].rearrange("(n p) l -> p n l", p=128)
                for dt in range(DT):
                    nc.sync.dma_start(out=xT_sb[:, dt, :], in_=xT_r[:, dt, :])
                nc.sync.dma_start(out=xTq_sb, in_=xTq_h[:].rearrange("(n p) l -> p n l", p=128))
                nc.sync.dma_start(out=wqkv_sb[:, :, 0:D], in_=wq_r[:, :, 0:D])
                nc.sync.dma_start(out=wqkv_sb[:, :, 2 * D:3 * D], in_=wq_r[:, :, 2 * D:3 * D])
                nc.sync.dma_start(out=wp_sb, in_=wp_h[:])
                bp_src = bp_h[:]
                nc.gpsimd.dma_start(
                    out=bp_sb,
                    in_=bass.AP(tensor=bp_src.tensor, offset=bp_src.offset,
                                ap=[[0, 128]] + list(bp_src.ap)),
                )

                D_ps = ps_d.tile([128, LQ], F32)

                def ktqt_block(kt):
                    # K^T c-tile: single-op evac with per-partition bias
                    for lc in range(4):
                        ps = ps_s.tile([128, JG, LQ], F32, tag="sps")
                        for dt in range(DT):
                            nc.tensor.matmul(
                                ps[:, 0, :],
                                wqkv_sb[:, dt, D + kt * 128:D + (kt + 1) * 128],
                                xT_sb[:, dt, lc * 512:(lc + 1) * 512],
                                start=(dt == 0), stop=(dt == DT - 1),
                            )
                        nc.vector.tensor_scalar_add(
                            KT_sb[:, kt, lc * 512:(lc + 1) * 512], ps[:, 0, :],
                            bias_sb[:, 6 + kt:7 + kt])
                    # Q^T c-tile: evac halves split into the per-head
                    # zero-padded layout (ScalarE / VectorE)
                    ps = ps_s.tile([128, JG, LQ], F32, tag="sps")
                    for dt in range(DT):
                        nc.tensor.matmul(
                            ps[:, 0, :],
                            wqkv_sb[:, dt, kt * 128:(kt + 1) * 128],
                            xTq_sb[:, dt, :],
                            start=(dt == 0), stop=(dt == DT - 1),
                        )
                    nc.scalar.activation(
                        QTz_sb[0:64, 2 * kt, :], ps[0:64, 0, :],
                        Act.Identity, bias=bias_sb[0:64, kt:kt + 1])
                    nc.vector.tensor_scalar_add(
                        QTz_sb[64:128, 2 * kt + 1, :], ps[64:128, 0, :],
                        bias_sb[64:128, kt:kt + 1])

                def v_block(lt):
                    for hf in range(2):
                        ps = ps_o.tile([128, LQ], F32, tag="ops")
                        for dt in range(DT):
                            nc.tensor.matmul(
                                ps[:, :384],
                                xT_sb[:, dt, lt * 128:(lt + 1) * 128],
                                wqkv_sb[:, dt, 2 * D + hf * 384:2 * D + (hf + 1) * 384],
                                start=(dt == 0), stop=(dt == DT - 1),
                            )
                        nc.vector.tensor_tensor(
                            V_sb[:, lt, 390 * hf:390 * hf + 390].rearrange(
                                "p (h c) -> p h c", c=VW)[:, :, 0:HD],
                            ps[:, :384].rearrange("p (h d) -> p h d", h=6),
                            bv_sb[:, hf * 384:(hf + 1) * 384].rearrange(
                                "p (h d) -> p h d", h=6),
                            Alu.add,
                        )

                def s_block(h):
                    # S^T[j, i] = sum_d K^T[d, j] Q^T[d, i]; zeros in QTz kill
                    # the neighboring head's K rows
                    PT = ptp.tile([128, LT, LQ], BF16, tag="PT")
                    for g in range(LT // JG):
                        sps = ps_s.tile([128, JG, LQ], F32, tag="sps")
                        for t in range(JG):
                            jt = JG * g + t
                            nc.tensor.matmul(
                                sps[:, t, :],
                                KT_sb[:, h // 2, jt * 128:(jt + 1) * 128],
                                QTz_sb[:, h, :],
                                start=True, stop=True,
                            )
                        nc.scalar.activation(
                            PT[:, JG * g:JG * (g + 1), :], sps, Act.Exp,
                            scale=SCALE)
                    return PT

                def out_block(h, PT):
                    # out^T = sum_j [..V_h|1..][j, :] P^T[j, i] through a
                    # 128-wide window: even heads ctx rows 0:64 + denom row 64,
                    # odd heads (window shifted 64 left) ctx rows 64:128 +
                    # denom row 63 (previous head's ones-column)
                    p0 = (h % 2) * 64
                    dr = 64 - (h % 2)
                    voff = h * VW - p0
                    ops = ps_o.tile([128, LQ], F32, tag="ops")
                    for jt in range(LT):
                        nc.tensor.matmul(
                            ops,
                            V_sb[:, jt, voff:voff + 128],
                            PT[:, jt, :],
                            start=(jt == 0), stop=(jt == LT - 1),
                        )
                    nc.vector.tensor_copy(
                        OT2_sb[p0:p0 + 64, h // 2, :], ops[p0:p0 + 64, :])
                    if h % 2 == 0:
                        nc.vector.tensor_copy(dst_sb[64:65, :], ops[64:65, :])
                    else:
                        nc.vector.tensor_copy(dst_sb[32:64, :], ops[32:64, :])
                    nc.tensor.matmul(
                        D_ps, eye_sb[:, h * 128:(h + 1) * 128], dst_sb,
                        start=(h == 0), stop=(h == H - 1),
                        skip_group_check=True,
                    )

                # normalization (reciprocal + bcast + mult)
                def normalize(h0, h1):
                    nc.vector.tensor_copy(Dsb[0:h1, :], D_ps[0:H, :][0:h1, :])
                    nc.vector.reciprocal(out=Rsb[0:h1, :], in_=Dsb[0:h1, :])
                    nc.vector.tensor_copy(R16[0:h1, :], Rsb[0:h1, :])
                    for h in range(h0, h1):
                        p0 = (h % 2) * 64
                        rb = ps_s.tile([128, LQ], F32, tag="sps")
                        nc.tensor.matmul(
                            rb, sel_sb[:, h * 128:(h + 1) * 128], R16,
                            start=True, stop=True)
                        nc.vector.tensor_tensor(
                            OT2_sb[p0:p0 + 64, h // 2, :],
                            OT2_sb[p0:p0 + 64, h // 2, :], rb[p0:p0 + 64, :],
                            Alu.mult)

                # ---- schedule: K^T/Q^T, first two heads' scores, V, rest ----
                for kt in range(DT):
                    ktqt_block(kt)
                PT0 = s_block(0)
                PT1 = s_block(1)
                for lt in range(LT):
                    v_block(lt)
                out_block(0, PT0)
                out_block(1, PT1)
                for h in range(2, H):
                    PT = s_block(h)
                    out_block(h, PT)
                normalize(0, H)

                # ---- projection, interleaved with normalization via deps ----
                with tc.tile_pool(name="yp", bufs=2) as yp:
                    y_r = y_h[:].rearrange("(n p) e -> p n e", p=128)
                    for ic in range(LQ // 128):
                        for eh in range(2):
                            ps = ps_o.tile([128, LQ], F32, tag="ops")
                            for pt in range(DT):
                                nc.tensor.matmul(
                                    ps[:, :384],
                                    OT2_sb[:, pt, ic * 128:(ic + 1) * 128],
                                    wp_sb[:, pt, eh * 384:(eh + 1) * 384],
                                    start=(pt == 0), stop=(pt == DT - 1),
                                )
                            yt = yp.tile([128, 384], F32)
                            nc.vector.tensor_tensor(
                                yt, ps[:, :384], bp_sb[:, eh * 384:(eh + 1) * 384],
                                Alu.add)
                            nc.sync.dma_start(
                                out=y_r[:, ic, eh * 384:(eh + 1) * 384], in_=yt)

    nc.finalize()
    return nc


def _get_nc():
    if "nc" not in _CACHED:
        _CACHED["nc"] = _build_nc()
    return _CACHED["nc"]


def _make_in_maps(x, Wqkv, bqkv, Wproj, bproj):
    bf16 = ml_dtypes.bfloat16
    x = np.asarray(x, dtype=np.float32)
    wqkv16 = np.ascontiguousarray(np.asarray(Wqkv, dtype=np.float32).astype(bf16))
    bqkv32 = np.ascontiguousarray(np.asarray(bqkv, dtype=np.float32))
    wp2 = np.ascontiguousarray(
        np.asarray(Wproj, dtype=np.float32).astype(bf16)
        .reshape(D // 128, 128, D).transpose(1, 0, 2))
    bp32 = np.ascontiguousarray(np.asarray(bproj, dtype=np.float32))
    selmat = np.zeros((H, H * 128), ml_dtypes.bfloat16)
    for h in range(H):
        selmat[h, h * 128:(h + 1) * 128] = 1.0
    eyemat = np.zeros((128, H * 128), ml_dtypes.bfloat16)
    for h in range(H):
        eyemat[64 - (h % 2), h * 128 + h] = 1.0

    xT = [np.ascontiguousarray(x[b].T.astype(bf16)) for b in range(B)]
    in_maps = []
    for c in range(N_CORES):
        b, s = c // 4, c % 4
        in_maps.append({
            "xT": xT[b],
            "xTq": np.ascontiguousarray(xT[b][:, s * LQ:(s + 1) * LQ]),
            "wqkv": wqkv16,
            "bqkv": bqkv32,
            "wproj2": wp2,
            "bproj": bp32,
            "selmat": selmat,
            "eyemat": eyemat,
        })
    return in_maps


def run(inputs, trace=False):
    """Run the SPMD kernel. Returns (full_output [2,2048,768] f32, BassKernelResults)."""
    from concourse.bass_utils import run_bass_kernel_spmd

    nc = _get_nc()
    in_maps = _make_in_maps(**inputs)
    res = run_bass_kernel_spmd(nc, in_maps, list(range(N_CORES)), trace=trace)
    out = np.empty((B, L, D), dtype=np.float32)
    for c in range(N_CORES):
        b, s = c // 4, c % 4
        out[b, s * LQ:(s + 1) * LQ, :] = res.results[c]["y"]
    return out, res


def kernel(**inputs) -> np.ndarray:
    return run(inputs)[0]


# revision 26
# speedup vs baseline: 1.0901x; 1.0167x over previous
"""Distributed multi-head attention kernel for 8 Trainium2 NeuronCores.

Problem: x[2,2048,768] @ Wqkv[768,2304] + bqkv -> 12-head attention -> @ Wproj + bproj.

Sharding: data-parallel over batch (2) x sequence-quarter (4) = 8 cores.
Each core computes K/V for its whole batch (redundant across the 4 cores
sharing a batch -- avoids collectives, which measured slower), attention +
projection for its own 512 query rows.

All matmuls are full-width 128-contract / 128-out (partial-array matmuls run
at half the PE clock):
- The per-head S^T matmul (contract = head_dim = 64) is zero-padded via a
  per-head Q^T buffer whose complementary partition half is zeroed (zeros in
  the streaming operand kill the neighboring head's K rows in the stationary
  operand).
- The attention-value matmul reads a 128-wide window of the packed
  [V_0|1|V_1|1|...] buffer. Even heads use window [h*65, h*65+128): context
  rows 0:64, denominator (ones-column) row 64. Odd heads shift the window 64
  left: context lands on rows 64:128 and the PREVIOUS head's ones-column
  lands on row 63 -- still all-ones, so it yields this head's denominator.
  Junk output rows are never read.
- Head pairs therefore assemble a fully-packed context^T [128, 6, 512], and
  the projection contracts 128 real rows per pair (6 matmuls per output tile,
  no padding).

Softmax runs without max-subtraction (scores are O(1) for this data regime).
All 12 head denominators are gathered into one PSUM tile via one-hot matmuls
(host-provided one-hots at rows 63/64), inverted in a single DVE reciprocal,
broadcast back via selector matmuls, and multiplied into the context right
before projection (projection is linear, so normalize-late is exact).

Engine balance: zero-fills run on GpSimd, K^T/V evacuation on VectorE, Q^T
evacuation split ScalarE/VectorE, exp on ScalarE in 2-bank groups, and the
first two heads' score matmuls are emitted before the V projection so the
ScalarE exp pipeline starts early.
"""

import numpy as np
import ml_dtypes

B = 2
L = 2048
D = 768
H = 12
HD = 64
SCALE = HD ** -0.5
N_CORES = 8
LQ = L // 4  # 512 query rows per core

_CACHED = {}


def _build_nc():
    import concourse.bass as bass
    import concourse.mybir as mybir
    import concourse.tile as tile
    from concourse import bacc

    F32 = mybir.dt.float32
    BF16 = mybir.dt.bfloat16
    Alu = mybir.AluOpType
    Act = mybir.ActivationFunctionType

    nc = bacc.Bacc(target_bir_lowering=False)

    xT_h = nc.declare_dram_parameter("xT", [D, L], BF16, isOutput=False)
    xTq_h = nc.declare_dram_parameter("xTq", [D, LQ], BF16, isOutput=False)
    wqkv_h = nc.declare_dram_parameter("wqkv", [D, 3 * D], BF16, isOutput=False)
    bqkv_h = nc.declare_dram_parameter("bqkv", [3 * D], F32, isOutput=False)
    wp_h = nc.declare_dram_parameter("wproj2", [128, D // 128, D], BF16, isOutput=False)
    sel_h = nc.declare_dram_parameter("selmat", [H, H * 128], BF16, isOutput=False)
    eye_h = nc.declare_dram_parameter("eyemat", [128, H * 128], BF16, isOutput=False)
    bp_h = nc.declare_dram_parameter("bproj", [D], F32, isOutput=False)
    y_h = nc.declare_dram_parameter("y", [LQ, D], F32, isOutput=True)

    DT = D // 128      # 6 tiles of the qkv contraction dim
    LT = L // 128      # 16 key tiles
    JG = 2             # j-tiles per exp group (psum banks per S tile)
    VW = 65            # V block width per head (64 ctx + 1 ones)
    VPAD = 11 * VW + 128  # pad V free dim so lhsT can read 128 cols

    with tile.TileContext(nc) as tc:
        with tc.tile_pool(name="persist", bufs=1) as pp:
            # persistent across the whole kernel
            KT_sb = pp.tile([128, DT, L], BF16)         # K^T, plain [c, l] layout
            QTz_sb = pp.tile([128, H, LQ], BF16)        # Q^T per head, other parity half zeroed
            V_sb = pp.tile([128, LT, VPAD + 5], BF16)   # [V_h | ones] blocks at h*65
            OT2_sb = pp.tile([128, DT, LQ], BF16)       # ctx^T per head PAIR (fully packed)
            bias_sb = pp.tile([128, 18], F32)
            bv_sb = pp.tile([128, D], F32)
            eye_sb = pp.tile([128, H * 128], BF16)      # gather one-hots at rows 63/64
            sel_sb = pp.tile([128, H * 128], BF16)      # bcast selectors on rows 0:12, rest zero
            R16 = pp.tile([128, LQ], BF16)              # bf16 1/denom, rows 12:128 zero
            dst_sb = pp.tile([128, LQ], BF16)           # denom staging, rows 63/64 live
            Rsb = pp.tile([128, LQ], F32)               # 1/denom on rows 0:12, rest zero
            Dsb = pp.tile([H, LQ], F32)

            # constants (zero-fills on the otherwise idle GpSimd engine)
            for h in range(H):
                nc.gpsimd.memset(QTz_sb[:, h, :], 0.0)
            nc.gpsimd.memset(dst_sb, 0.0)
            nc.gpsimd.memset(Rsb, 0.0)
            nc.gpsimd.memset(R16, 0.0)
            nc.vector.memset(sel_sb, 0.0)
            nc.sync.dma_start(out=sel_sb[0:H, :], in_=sel_h[:])
            nc.sync.dma_start(out=eye_sb, in_=eye_h[:])
            for h in range(H):
                nc.vector.memset(V_sb[:, :, h * VW + HD:h * VW + HD + 1], 1.0)

            nc.sync.dma_start(out=bias_sb, in_=bqkv_h[:].rearrange("(n p) -> p n", p=128))
            bv_src = bqkv_h[2 * D:3 * D]
            nc.gpsimd.dma_start(
                out=bv_sb,
                in_=bass.AP(tensor=bv_src.tensor, offset=bv_src.offset,
                            ap=[[0, 128]] + list(bv_src.ap)),
            )
            with (
                tc.tile_pool(name="loadp", bufs=1) as lp,
                tc.tile_pool(name="ps_s", bufs=2, space="PSUM") as ps_s,
                tc.tile_pool(name="ps_o", bufs=3, space="PSUM") as ps_o,
                tc.tile_pool(name="ps_d", bufs=1, space="PSUM") as ps_d,
                tc.tile_pool(name="ptp", bufs=3) as ptp,
            ):
                xT_sb = lp.tile([128, DT, L], BF16)
                xTq_sb = lp.tile([128, DT, LQ], BF16)
                wqkv_sb = lp.tile([128, DT, 3 * D], BF16)
                wp_sb = lp.tile([128, DT, D], BF16)
                bp_sb = lp.tile([128, D], F32)

                wq_r = wqkv_h[:].rearrange("(n p) c -> p n c", p=128)
                nc.sync.dma_start(out=wqkv_sb[:, :, D:2 * D], in_=wq_r[:, :, D:2 * D])
                xT_r = xT_h[:].rearrange("(n p) l -> p n l", p=128)
                for dt in range(DT):
                    nc.sync.dma_start(out=xT_sb[:, dt, :], in_=xT_r[:, dt, :])
                nc.sync.dma_start(out=xTq_sb, in_=xTq_h[:].rearrange("(n p) l -> p n l", p=128))
                nc.sync.dma_start(out=wqkv_sb[:, :, 0:D], in_=wq_r[:, :, 0:D])
                nc.sync.dma_start(out=wqkv_sb[:, :, 2 * D:3 * D], in_=wq_r[:, :, 2 * D:3 * D])
                nc.sync.dma_start(out=wp_sb, in_=wp_h[:])
                bp_src = bp_h[:]
                nc.gpsimd.dma_start(
                    out=bp_sb,
                    in_=bass.AP(tensor=bp_src.tensor, offset=bp_src.offset,
                                ap=[[0, 128]] + list(bp_src.ap)),
                )

                D_ps = ps_d.tile([128, LQ], F32)

                def ktqt_block(kt):
                    # K^T c-tile: single-op evac with per-partition bias
                    for lc in range(4):
                        ps = ps_s.tile([128, JG, LQ], F32, tag="sps")
                        for dt in range(DT):
                            nc.tensor.matmul(
                                ps[:, 0, :],
                                wqkv_sb[:, dt, D + kt * 128:D + (kt + 1) * 128],
                                xT_sb[:, dt, lc * 512:(lc + 1) * 512],
                                start=(dt == 0), stop=(dt == DT - 1),
                            )
                        nc.vector.tensor_scalar_add(
                            KT_sb[:, kt, lc * 512:(lc + 1) * 512], ps[:, 0, :],
                            bias_sb[:, 6 + kt:7 + kt])
                    # Q^T c-tile: evac halves split into the per-head
                    # zero-padded layout (ScalarE / VectorE)
                    ps = ps_s.tile([128, JG, LQ], F32, tag="sps")
                    for dt in range(DT):
                        nc.tensor.matmul(
                            ps[:, 0, :],
                            wqkv_sb[:, dt, kt * 128:(kt + 1) * 128],
                            xTq_sb[:, dt, :],
                            start=(dt == 0), stop=(dt == DT - 1),
                        )
                    nc.vector.tensor_scalar_add(
                        QTz_sb[0:64, 2 * kt, :], ps[0:64, 0, :],
                        bias_sb[0:64, kt:kt + 1])
                    nc.vector.tensor_scalar_add(
                        QTz_sb[64:128, 2 * kt + 1, :], ps[64:128, 0, :],
                        bias_sb[64:128, kt:kt + 1])

                def v_block(lt):
                    for hf in range(2):
                        ps = ps_o.tile([128, LQ], F32, tag="ops")
                        for dt in range(DT):
                            nc.tensor.matmul(
                                ps[:, :384],
                                xT_sb[:, dt, lt * 128:(lt + 1) * 128],
                                wqkv_sb[:, dt, 2 * D + hf * 384:2 * D + (hf + 1) * 384],
                                start=(dt == 0), stop=(dt == DT - 1),
                            )
                        nc.vector.tensor_tensor(
                            V_sb[:, lt, 390 * hf:390 * hf + 390].rearrange(
                                "p (h c) -> p h c", c=VW)[:, :, 0:HD],
                            ps[:, :384].rearrange("p (h d) -> p h d", h=6),
                            bv_sb[:, hf * 384:(hf + 1) * 384].rearrange(
                                "p (h d) -> p h d", h=6),
                            Alu.add,
                        )

                def s_block(h):
                    # S^T[j, i] = sum_d K^T[d, j] Q^T[d, i]; zeros in QTz kill
                    # the neighboring head's K rows
                    PT = ptp.tile([128, LT, LQ], BF16, tag="PT")
                    for g in range(LT // JG):
                        sps = ps_s.tile([128, JG, LQ], F32, tag="sps")
                        for t in range(JG):
                            jt = JG * g + t
                            nc.tensor.matmul(
                                sps[:, t, :],
                                KT_sb[:, h // 2, jt * 128:(jt + 1) * 128],
                                QTz_sb[:, h, :],
                                start=True, stop=True,
                            )
                        nc.scalar.activation(
                            PT[:, JG * g:JG * (g + 1), :], sps, Act.Exp,
                            scale=SCALE)
                    return PT

                def out_block(h, PT):
                    # out^T = sum_j [..V_h|1..][j, :] P^T[j, i] through a
                    # 128-wide window: even heads ctx rows 0:64 + denom row 64,
                    # odd heads (window shifted 64 left) ctx rows 64:128 +
                    # denom row 63 (previous head's ones-column)
                    p0 = (h % 2) * 64
                    dr = 64 - (h % 2)
                    voff = h * VW - p0
                    ops = ps_o.tile([128, LQ], F32, tag="ops")
                    for jt in range(LT):
                        nc.tensor.matmul(
                            ops,
                            V_sb[:, jt, voff:voff + 128],
                            PT[:, jt, :],
                            start=(jt == 0), stop=(jt == LT - 1),
                        )
                    nc.vector.tensor_copy(
                        OT2_sb[p0:p0 + 64, h // 2, :], ops[p0:p0 + 64, :])
                    if h % 2 == 0:
                        nc.vector.tensor_copy(dst_sb[64:65, :], ops[64:65, :])
                    else:
                        nc.vector.tensor_copy(dst_sb[32:64, :], ops[32:64, :])
                    nc.tensor.matmul(
                        D_ps, eye_sb[:, h * 128:(h + 1) * 128], dst_sb,
                        start=(h == 0), stop=(h == H - 1),
                        skip_group_check=True,
                    )

                # normalization (reciprocal + bcast + mult)
                def normalize(h0, h1):
                    nc.vector.tensor_copy(Dsb[0:h1, :], D_ps[0:H, :][0:h1, :])
                    nc.vector.reciprocal(out=Rsb[0:h1, :], in_=Dsb[0:h1, :])
                    nc.vector.tensor_copy(R16[0:h1, :], Rsb[0:h1, :])
                    for h in range(h0, h1):
                        p0 = (h % 2) * 64
                        rb = ps_s.tile([128, LQ], F32, tag="sps")
                        nc.tensor.matmul(
                            rb, sel_sb[:, h * 128:(h + 1) * 128], R16,
                            start=True, stop=True)
                        nc.vector.tensor_tensor(
                            OT2_sb[p0:p0 + 64, h // 2, :],
                            OT2_sb[p0:p0 + 64, h // 2, :], rb[p0:p0 + 64, :],
                            Alu.mult)

                # ---- schedule: scores for heads 0-2 first (feeds ScalarE
                # from ~20us), V while ScalarE grinds, then a 3-deep
                # score/context pipeline with lazy K/Q c-tile emission ----
                ktqt_block(0)
                pending = [(0, s_block(0)), (1, s_block(1))]
                ktqt_block(1)
                pending.append((2, s_block(2)))
                for lt in range(LT):
                    v_block(lt)
                nexth = 3
                while pending:
                    h, PT = pending.pop(0)
                    out_block(h, PT)
                    if nexth < H:
                        if nexth % 2 == 0:
                            ktqt_block(nexth // 2)
                        pending.append((nexth, s_block(nexth)))
                        nexth += 1
                normalize(0, H)

                # ---- projection, interleaved with normalization via deps ----
                with tc.tile_pool(name="yp", bufs=2) as yp:
                    y_r = y_h[:].rearrange("(n p) e -> p n e", p=128)
                    for ic in range(LQ // 128):
                        for eh in range(2):
                            ps = ps_o.tile([128, LQ], F32, tag="ops")
                            for pt in range(DT):
                                nc.tensor.matmul(
                                    ps[:, :384],
                                    OT2_sb[:, pt, ic * 128:(ic + 1) * 128],
                                    wp_sb[:, pt, eh * 384:(eh + 1) * 384],
                                    start=(pt == 0), stop=(pt == DT - 1),
                                )
                            yt = yp.tile([128, 384], F32)
                            nc.vector.tensor_tensor(
                                yt, ps[:, :384], bp_sb[:, eh * 384:(eh + 1) * 384],
                                Alu.add)
                            nc.sync.dma_start(
                                out=y_r[:, ic, eh * 384:(eh + 1) * 384], in_=yt)

    nc.finalize()
    return nc


def _get_nc():
    if "nc" not in _CACHED:
        _CACHED["nc"] = _build_nc()
    return _CACHED["nc"]


def _make_in_maps(x, Wqkv, bqkv, Wproj, bproj):
    bf16 = ml_dtypes.bfloat16
    x = np.asarray(x, dtype=np.float32)
    wqkv16 = np.ascontiguousarray(np.asarray(Wqkv, dtype=np.float32).astype(bf16))
    bqkv32 = np.ascontiguousarray(np.asarray(bqkv, dtype=np.float32))
    wp2 = np.ascontiguousarray(
        np.asarray(Wproj, dtype=np.float32).astype(bf16)
        .reshape(D // 128, 128, D).transpose(1, 0, 2))
    bp32 = np.ascontiguousarray(np.asarray(bproj, dtype=np.float32))
    selmat = np.zeros((H, H * 128), ml_dtypes.bfloat16)
    for h in range(H):
        selmat[h, h * 128:(h + 1) * 128] = 1.0
    eyemat = np.zeros((128, H * 128), ml_dtypes.bfloat16)
    for h in range(H):
        eyemat[64 - (h % 2), h * 128 + h] = 1.0

    xT = [np.ascontiguousarray(x[b].T.astype(bf16)) for b in range(B)]
    in_maps = []
    for c in range(N_CORES):
        b, s = c // 4, c % 4
        in_maps.append({
            "xT": xT[b],
            "xTq": np.ascontiguousarray(xT[b][:, s * LQ:(s + 1) * LQ]),
            "wqkv": wqkv16,
            "bqkv": bqkv32,
            "wproj2": wp2,
            "bproj": bp32,
            "selmat": selmat,
            "eyemat": eyemat,
        })
    return in_maps


def run(inputs, trace=False):
    """Run the SPMD kernel. Returns (full_output [2,2048,768] f32, BassKernelResults)."""
    from concourse.bass_utils import run_bass_kernel_spmd

    nc = _get_nc()
    in_maps = _make_in_maps(**inputs)
    res = run_bass_kernel_spmd(nc, in_maps, list(range(N_CORES)), trace=trace)
    out = np.empty((B, L, D), dtype=np.float32)
    for c in range(N_CORES):
        b, s = c // 4, c % 4
        out[b, s * LQ:(s + 1) * LQ, :] = res.results[c]["y"]
    return out, res


def kernel(**inputs) -> np.ndarray:
    return run(inputs)[0]
